# Initial kernel scaffold
#
"""Trainium2 Bass kernel v4: software-pipelined chain-DAG generator MLP.

Delta vs v3 (267 us): chunks take 14 pipeline steps instead of 16 by fusing
the relus of banks (12,14) and (13,15) into single [128,512] ops (those
banks are chain-complete when first needed); the two b2 bias matmuls are
folded into the DVE final (scalar_tensor_tensor reads only bank B from
PSUM, which walrus allows); base matmuls are emitted just-in-time (step
a-1) so each pipeline slot holds only one pair bank per tile pool; x DMAs
are batched two chunks per transfer to halve the sync-engine issue cost
(~580 ns per dma_start regardless of size).

Core structure (measured on HW): bf16 everywhere, PE 64x128 row tiling
with T0 = SBUF partitions 0:64 / T8 = 64:128 running concurrently, PSUM
pools pinned per tile (a bank written by both tiles crashes), same-parity
node pairs per bank so every accumulation group stays on one tile, three
pipeline slots staggered 5 nodes apart walking independent chunks' chains,
collects deferred until a chunk's chain completes and trickled at 4 ops
per global step to keep the PE dense and warm.
"""

import threading

import numpy as np
import ml_dtypes

import concourse.bacc as bacc
import concourse.mybir as mybir
from concourse.bass_utils import run_bass_kernel_spmd
from concourse.tile import TileContext

N_CORES = 8
B_FULL = 131072
B_S = B_FULL // N_CORES  # 16384
CHUNK = 512
I_DIM = 16
I_STEPS = 14  # steps per chunk: relus of 12..15 are fused into steps 12/13
NSLOT = 3
STAG = 5

F32 = mybir.dt.float32
BF16 = mybir.dt.bfloat16
BF16_NP = ml_dtypes.bfloat16

PAIRS = [(1, 3), (5, 7), (9, 11), (13, 15), (0, 2), (4, 6), (8, 10), (12, 14)]
PAIR_OF = {}
for _p, (_a, _b) in enumerate(PAIRS):
    PAIR_OF[_a] = (_p, 0)
    PAIR_OF[_b] = (_p, 1)


def _node_part0(i: int) -> int:
    return 0 if i % 2 == 0 else 64


# h-tile partition offset per node AFTER the fused-relu layout:
#   fusedA (bank (12,14)): h12 at 0:64, h14 at 64:128
#   fusedB (bank (13,15)): h13 at 0:64, h15 at 64:128
def _h_part0(i: int) -> int:
    if i == 13:
        return 0
    if i == 14:
        return 64
    return _node_part0(i)


def build_nc(b_s: int = B_S, chunk: int = CHUNK, num_devices: int = N_CORES):
    assert b_s % chunk == 0
    n_chunks = b_s // chunk
    assert n_chunks % 2 == 0

    nc = bacc.Bacc(
        "TRN2", target_bir_lowering=False, debug=False, num_devices=num_devices
    )

    xt_d = nc.dram_tensor("XT", [48, 8 * b_s], BF16, kind="ExternalInput").ap()
    px_d = nc.dram_tensor("PX", [128, 128 * 8], BF16, kind="ExternalInput").ap()
    mc_d = nc.dram_tensor("MC", [128, 128 * 13], BF16, kind="ExternalInput").ap()
    cl_d = nc.dram_tensor("CLW", [128, 128 * 16], BF16, kind="ExternalInput").ap()
    b2_d = nc.dram_tensor("B2", [16, 1], F32, kind="ExternalInput").ap()
    out_d = nc.dram_tensor("OUT", [16, b_s], F32, kind="ExternalOutput").ap()

    relu_f = mybir.ActivationFunctionType.Relu
    copy_f = mybir.ActivationFunctionType.Copy
    add_op = mybir.AluOpType.add

    with TileContext(nc) as tc:
        with (
            tc.tile_pool(name="consts", bufs=1) as cpool,
            tc.tile_pool(name="xs", bufs=24) as xpool,
            tc.tile_pool(name="hbuf", bufs=90) as hpool,
            tc.tile_pool(name="obuf", bufs=6) as opool,
            tc.tile_pool(name="pt0", bufs=3, space="PSUM") as pool_t0,
            tc.tile_pool(name="pt8", bufs=3, space="PSUM") as pool_t8,
            tc.tile_pool(name="qt0", bufs=1, space="PSUM") as qpool_t0,
            tc.tile_pool(name="qt8", bufs=1, space="PSUM") as qpool_t8,
        ):
            px_t = cpool.tile([128, 128 * 8], BF16)
            nc.sync.dma_start(out=px_t[:, :], in_=px_d[:, :])
            mc_t = cpool.tile([128, 128 * 13], BF16)
            nc.sync.dma_start(out=mc_t[:, :], in_=mc_d[:, :])
            cl_t = cpool.tile([128, 128 * 16], BF16)
            nc.sync.dma_start(out=cl_t[:, :], in_=cl_d[:, :])
            b2_t = cpool.tile([16, 1], F32)
            nc.sync.dma_start(out=b2_t[:, :], in_=b2_d[:, :])

            class ChunkState:
                def __init__(self, c):
                    self.c = c
                    self.banks = [None] * 8
                    self.h = [None] * I_DIM  # (tile, part0) per node

            chunks = [ChunkState(c) for c in range(n_chunks)]
            xgroups = {}  # (g, p) -> tile [128, 2*chunk]

            def emit_xdma_group(g, p):
                part0 = 0 if p < 4 else 64
                x_t = xpool.tile(
                    [128, 2 * chunk], BF16, tag="x", name=f"x_{g}_{p}"
                )
                xgroups[(g, p)] = x_t
                c0 = 2 * g * chunk
                nc.sync.dma_start(
                    out=x_t[part0 : part0 + 48, :],
                    in_=xt_d[:, p * b_s + c0 : p * b_s + c0 + 2 * chunk],
                )

            def emit_base(c, p):
                st = chunks[c]
                part0 = 0 if p < 4 else 64
                pool = pool_t0 if p < 4 else pool_t8
                bank = pool.tile([128, chunk], F32, tag="bank", name=f"bank_{c}_{p}")
                st.banks[p] = bank
                x_t = xgroups[(c // 2, p)]
                h0 = (c % 2) * chunk
                nc.tensor.matmul(
                    out=bank[:, :],
                    lhsT=px_t[part0 : part0 + 48, 128 * p : 128 * (p + 1)],
                    rhs=x_t[part0 : part0 + 48, h0 : h0 + chunk],
                    start=True,
                    stop=False,
                    skip_group_check=True,
                )

            def emit_relu(c, i):
                """Relu for pipeline step i (i in 0..13); steps 12/13 are the
                fused [128,512] relus of banks (12,14) and (13,15)."""
                st = chunks[c]
                if i == 12:  # fused bank (12,14)
                    p = PAIR_OF[12][0]
                    h = hpool.tile([128, chunk], BF16, tag="h", name=f"hA_{c}")
                    nc.scalar.activation(h[:, :], st.banks[p][:, :], relu_f)
                    st.h[12] = h
                    st.h[14] = h
                    return
                if i == 13:  # fused bank (13,15)
                    p = PAIR_OF[13][0]
                    h = hpool.tile([128, chunk], BF16, tag="h", name=f"hB_{c}")
                    nc.vector.tensor_scalar_max(
                        out=h[:, :], in0=st.banks[p][:, :], scalar1=0.0
                    )
                    st.h[13] = h
                    st.h[15] = h
                    return
                p, r = PAIR_OF[i]
                part0 = _node_part0(i)
                h = hpool.tile([128, chunk], BF16, tag="h", name=f"h_{c}_{i}")
                st.h[i] = h
                src = st.banks[p][64 * r : 64 * (r + 1), :]
                dst = h[part0 : part0 + 64, :]
                if i % 2 == 0:
                    nc.scalar.activation(dst, src, relu_f)
                else:
                    nc.vector.tensor_scalar_max(out=dst, in0=src, scalar1=0.0)

            def emit_chain(c, i):
                if i > 12:
                    return
                st = chunks[c]
                part0 = _h_part0(i)
                cp, cr = PAIR_OF[i + 1]
                ca, cb = PAIRS[cp]
                closes = (i + 1 == cb) or (i + 1 == ca and not (1 <= cb <= 13))
                nc.tensor.matmul(
                    out=st.banks[cp][:, :],
                    lhsT=mc_t[part0 : part0 + 64, 128 * i : 128 * (i + 1)],
                    rhs=st.h[i][part0 : part0 + 64, :],
                    start=False,
                    stop=closes,
                    skip_group_check=True,
                )

            A_NODES = {0, 2, 4, 6, 8, 10, 12, 13}

            def make_collect_ops(c):
                st = chunks[c]
                bank_a = qpool_t0.tile([128, chunk], F32, tag="bA", name=f"bA_{c}")
                bank_b = qpool_t8.tile([128, chunk], F32, tag="bB", name=f"bB_{c}")

                def collect(i):
                    part0 = _h_part0(i)
                    in_a = i in A_NODES
                    ob = bank_a if in_a else bank_b
                    nc.tensor.matmul(
                        out=ob[:, :],
                        lhsT=cl_t[part0 : part0 + 64, 128 * i : 128 * (i + 1)],
                        rhs=st.h[i][part0 : part0 + 64, :],
                        start=(i <= 1),
                        stop=(i == 13) if in_a else (i == 15),
                        skip_group_check=True,
                    )
                    st.h[i] = None

                ops = [lambda i=i: collect(i) for i in range(I_DIM)]

                def finals():
                    o_a = opool.tile([16, chunk], F32, tag="oa", name=f"oa_{c}")
                    nc.scalar.activation(o_a[:, :], bank_a[0:16, :], copy_f)
                    o_t = opool.tile([16, chunk], F32, tag="o", name=f"o_{c}")
                    nc.vector.scalar_tensor_tensor(
                        out=o_t[:, :],
                        in0=bank_b[0:16, :],
                        scalar=b2_t[:, 0:1],
                        in1=o_a[:, :],
                        op0=add_op,
                        op1=add_op,
                    )
                    c0 = c * chunk
                    nc.sync.dma_start(out=out_d[:, c0 : c0 + chunk], in_=o_t[:, :])

                ops.append(finals)
                return ops

            # ---------- pipeline ----------
            # base for pair containing lower node a lands at step a-1
            base_step = {p: a - 1 for p, (a, b) in enumerate(PAIRS)}
            pending = []

            def slot_pos(k, T):
                q = T - STAG * k
                if q < 0:
                    return None, None
                j, i = divmod(q, I_STEPS)
                c = NSLOT * j + k
                return (c, i) if c < n_chunks else (None, None)

            # prologue: group-0 x DMAs in first-use order, while ~24 scratch
            # matmuls (garbage data, never read) warm the PE's HAM clock gate
            for p in (4, 0, 5, 1, 6, 2, 7, 3):
                emit_xdma_group(0, p)
            ws0 = pool_t0.tile([128, chunk], F32, tag="bank", name="warm0")
            ws8 = pool_t8.tile([128, chunk], F32, tag="bank", name="warm8")
            for w in range(24):
                nc.tensor.matmul(
                    out=(ws0 if w % 2 == 0 else ws8)[:, :],
                    lhsT=px_t[(w % 2) * 64 : (w % 2) * 64 + 48, 0:128],
                    rhs=px_t[(w % 2) * 64 : (w % 2) * 64 + 48, 0:chunk],
                    start=True, stop=True, skip_group_check=True,
                )

            max_T = I_STEPS * ((n_chunks + NSLOT - 1) // NSLOT) + STAG * NSLOT + 8
            for T in range(max_T):
                for k in range(NSLOT):
                    c, i = slot_pos(k, T)
                    if c is None:
                        continue
                    # x DMA prefetch: during even chunk c, step 6..13 emit
                    # group c//2 + 2's DMAs (two pairs per step); group 1
                    # lands during chunk 0 steps 2..5 (deferred from prologue)
                    if c == 0 and 2 <= i < 6 and n_chunks > 2:
                        pa, pb = ((4, 0), (5, 1), (6, 2), (7, 3))[i - 2]
                        emit_xdma_group(1, pa)
                        emit_xdma_group(1, pb)
                    if c % 2 == 0 and 6 <= i < 10:
                        g = c // 2 + 2
                        if 2 * g < n_chunks:
                            for p in (2 * (i - 6), 2 * (i - 6) + 1):
                                emit_xdma_group(g, p)
                    # JIT bases for this chunk / the slot's next chunk
                    if i == 0 and c < NSLOT:
                        for p in range(8):
                            if base_step[p] < 0:
                                emit_base(c, p)
                    for p in range(8):
                        if base_step[p] == i:
                            emit_base(c, p)
                        nxt = c + NSLOT
                        if nxt < n_chunks and base_step[p] < 0 \
                                and i == base_step[p] + I_STEPS:
                            emit_base(nxt, p)
                    emit_relu(c, i)
                    # paced fill: ~1 ready collect mm per slot turn keeps the
                    # PE busy through the relu->chain wait without draining
                    # the backlog dry (supply is ~3.6 ops/global step)
                    npop = 3 if len(pending) > 26 else (
                        2 if len(pending) > 12 else (1 if pending else 0))
                    for _ in range(npop):
                        if pending:
                            pending.pop(0)()
                    emit_chain(c, i)
                    if i == I_STEPS - 1:
                        pending.extend(make_collect_ops(c))
            while pending:
                pending.pop(0)()

    nc.compile()
    return nc


def prep_weights(noise_d, mu, sigma, Wc, W1, b1, W2, b2):
    theta = mu + np.log1p(np.exp(sigma)) * noise_d  # [4, 256]
    w_p = W1[:, 48, :]  # [16, 64]
    b1e = b1.copy()
    for i in range(1, 14):
        b1e[i] = b1[i] + w_p[i] * b2[i - 1]

    px = np.zeros((128, 128 * 8), np.float32)
    for p, (a, b) in enumerate(PAIRS):
        part0 = 0 if p < 4 else 64
        for r, node in enumerate((a, b)):
            cols = slice(128 * p + 64 * r, 128 * p + 64 * (r + 1))
            px[part0 + 0 : part0 + 10, cols] = (
                Wc[:, 16 * node : 16 * (node + 1)] @ W1[node, 0:16, :]
            )
            px[part0 + 10 : part0 + 14, cols] = (
                theta[:, 16 * node : 16 * (node + 1)] @ W1[node, 16:32, :]
            )
            px[part0 + 14, cols] = b1e[node]
            px[part0 + 16 + 16 * r : part0 + 32 + 16 * r, cols] = W1[node, 32:48, :]

    mc = np.zeros((128, 128 * 13), np.float32)
    for i in range(13):
        part0 = 0 if i % 2 == 0 else 64  # h position of parent i (i <= 12)
        cp, cr = PAIR_OF[i + 1]
        c0 = 128 * i + 64 * cr
        mc[part0 : part0 + 64, c0 : c0 + 64] = np.outer(W2[i], w_p[i + 1])

    cl = np.zeros((128, 128 * 16), np.float32)
    for i in range(16):
        part0 = 0 if i in (13,) or (i % 2 == 0 and i != 14) else 64
        cl[part0 : part0 + 64, 128 * i + i] = W2[i]

    return {
        "PX": px.astype(BF16_NP),
        "MC": mc.astype(BF16_NP),
        "CLW": cl.astype(BF16_NP),
        "B2": b2.reshape(16, 1).astype(np.float32),
    }


def prep_core_inputs(noise, input_c, input_d, c, b_s: int = B_S):
    b0, b1_ = c * b_s, (c + 1) * b_s
    s = np.zeros((16, b_s), np.float32)
    s[0:10] = input_c[b0:b1_].T
    s[10:14] = input_d[b0:b1_].T
    s[14] = 1.0
    nT = noise[b0:b1_].T
    xt = np.empty((48, 8 * b_s), np.float32)
    for p, (a, b) in enumerate(PAIRS):
        cols = slice(p * b_s, (p + 1) * b_s)
        xt[0:16, cols] = s
        xt[16:32, cols] = nT[16 * a : 16 * (a + 1)]
        xt[32:48, cols] = nT[16 * b : 16 * (b + 1)]
    return {"XT": xt.astype(BF16_NP)}


_NC_LOCK = threading.Lock()
_NC_CACHE = {}


def _get_nc():
    with _NC_LOCK:
        if "nc" not in _NC_CACHE:
            _NC_CACHE["nc"] = build_nc()
        return _NC_CACHE["nc"]


def kernel(noise, input_c, input_d, noise_d, mu, sigma, Wc, W1, b1, W2, b2):
    noise = np.asarray(noise, np.float32)
    input_c = np.asarray(input_c, np.float32)
    input_d = np.asarray(input_d, np.float32)
    w = prep_weights(
        np.asarray(noise_d, np.float32),
        np.asarray(mu, np.float32),
        np.asarray(sigma, np.float32),
        np.asarray(Wc, np.float32),
        np.asarray(W1, np.float32),
        np.asarray(b1, np.float32),
        np.asarray(W2, np.float32),
        np.asarray(b2, np.float32),
    )
    in_maps = []
    for c in range(N_CORES):
        m = prep_core_inputs(noise, input_c, input_d, c)
        m.update(w)
        in_maps.append(m)

    nc = _get_nc()
    res = run_bass_kernel_spmd(nc, in_maps, list(range(N_CORES)))
    out = np.concatenate(
        [res.results[c]["OUT"].T for c in range(N_CORES)], axis=0
    )
    return np.ascontiguousarray(out, np.float32)



# revision 1
# speedup vs baseline: 1.0495x; 1.0495x over previous
"""Trainium2 Bass kernel v4: software-pipelined chain-DAG generator MLP.

Delta vs v3 (267 us): chunks take 14 pipeline steps instead of 16 by fusing
the relus of banks (12,14) and (13,15) into single [128,512] ops (those
banks are chain-complete when first needed); the two b2 bias matmuls are
folded into the DVE final (scalar_tensor_tensor reads only bank B from
PSUM, which walrus allows); base matmuls are emitted just-in-time (step
a-1) so each pipeline slot holds only one pair bank per tile pool; x DMAs
are batched two chunks per transfer to halve the sync-engine issue cost
(~580 ns per dma_start regardless of size).

Core structure (measured on HW): bf16 everywhere, PE 64x128 row tiling
with T0 = SBUF partitions 0:64 / T8 = 64:128 running concurrently, PSUM
pools pinned per tile (a bank written by both tiles crashes), same-parity
node pairs per bank so every accumulation group stays on one tile, three
pipeline slots staggered 5 nodes apart walking independent chunks' chains,
collects deferred until a chunk's chain completes and trickled at 4 ops
per global step to keep the PE dense and warm.
"""

import threading

import numpy as np
import ml_dtypes

import concourse.bacc as bacc
import concourse.mybir as mybir
from concourse.bass_utils import run_bass_kernel_spmd
from concourse.tile import TileContext

N_CORES = 8
B_FULL = 131072
B_S = B_FULL // N_CORES  # 16384
CHUNK = 512
I_DIM = 16
I_STEPS = 14  # steps per chunk: relus of 12..15 are fused into steps 12/13
NSLOT = 3
STAG = 5

F32 = mybir.dt.float32
BF16 = mybir.dt.bfloat16
BF16_NP = ml_dtypes.bfloat16

PAIRS = [(1, 3), (5, 7), (9, 11), (13, 15), (0, 2), (4, 6), (8, 10), (12, 14)]
PAIR_OF = {}
for _p, (_a, _b) in enumerate(PAIRS):
    PAIR_OF[_a] = (_p, 0)
    PAIR_OF[_b] = (_p, 1)


def _node_part0(i: int) -> int:
    return 0 if i % 2 == 0 else 64


# h-tile partition offset per node AFTER the fused-relu layout:
#   fusedA (bank (12,14)): h12 at 0:64, h14 at 64:128
#   fusedB (bank (13,15)): h13 at 0:64, h15 at 64:128
def _h_part0(i: int) -> int:
    if i == 13:
        return 0
    if i == 14:
        return 64
    return _node_part0(i)


def build_nc(b_s: int = B_S, chunk: int = CHUNK, num_devices: int = N_CORES):
    assert b_s % chunk == 0
    n_chunks = b_s // chunk
    assert n_chunks % 2 == 0

    nc = bacc.Bacc(
        "TRN2", target_bir_lowering=False, debug=False, num_devices=num_devices
    )

    xt_d = nc.dram_tensor("XT", [48, 8 * b_s], BF16, kind="ExternalInput").ap()
    px_d = nc.dram_tensor("PX", [128, 128 * 8], BF16, kind="ExternalInput").ap()
    mc_d = nc.dram_tensor("MC", [128, 128 * 13], BF16, kind="ExternalInput").ap()
    cl_d = nc.dram_tensor("CLW", [128, 128 * 16], BF16, kind="ExternalInput").ap()
    b2_d = nc.dram_tensor("B2", [16, 1], F32, kind="ExternalInput").ap()
    out_d = nc.dram_tensor("OUT", [16, b_s], F32, kind="ExternalOutput").ap()

    relu_f = mybir.ActivationFunctionType.Relu
    copy_f = mybir.ActivationFunctionType.Copy
    add_op = mybir.AluOpType.add

    with TileContext(nc) as tc:
        with (
            tc.tile_pool(name="consts", bufs=1) as cpool,
            tc.tile_pool(name="xs", bufs=24) as xpool,
            tc.tile_pool(name="hbuf", bufs=90) as hpool,
            tc.tile_pool(name="obuf", bufs=6) as opool,
            tc.tile_pool(name="pt0", bufs=3, space="PSUM") as pool_t0,
            tc.tile_pool(name="pt8", bufs=3, space="PSUM") as pool_t8,
            tc.tile_pool(name="qt0", bufs=1, space="PSUM") as qpool_t0,
            tc.tile_pool(name="qt8", bufs=1, space="PSUM") as qpool_t8,
        ):
            px_t = cpool.tile([128, 128 * 8], BF16)
            nc.sync.dma_start(out=px_t[:, :], in_=px_d[:, :])
            mc_t = cpool.tile([128, 128 * 13], BF16)
            nc.sync.dma_start(out=mc_t[:, :], in_=mc_d[:, :])
            cl_t = cpool.tile([128, 128 * 16], BF16)
            nc.sync.dma_start(out=cl_t[:, :], in_=cl_d[:, :])
            b2_t = cpool.tile([16, 1], F32)
            nc.sync.dma_start(out=b2_t[:, :], in_=b2_d[:, :])

            class ChunkState:
                def __init__(self, c):
                    self.c = c
                    self.banks = [None] * 8
                    self.h = [None] * I_DIM  # (tile, part0) per node

            chunks = [ChunkState(c) for c in range(n_chunks)]
            xgroups = {}  # (g, p) -> tile [128, 2*chunk]

            def emit_xdma_group(g, p):
                part0 = 0 if p < 4 else 64
                x_t = xpool.tile(
                    [128, 2 * chunk], BF16, tag="x", name=f"x_{g}_{p}"
                )
                xgroups[(g, p)] = x_t
                c0 = 2 * g * chunk
                nc.sync.dma_start(
                    out=x_t[part0 : part0 + 48, :],
                    in_=xt_d[:, p * b_s + c0 : p * b_s + c0 + 2 * chunk],
                )

            def emit_base(c, p):
                st = chunks[c]
                part0 = 0 if p < 4 else 64
                pool = pool_t0 if p < 4 else pool_t8
                bank = pool.tile([128, chunk], F32, tag="bank", name=f"bank_{c}_{p}")
                st.banks[p] = bank
                x_t = xgroups[(c // 2, p)]
                h0 = (c % 2) * chunk
                nc.tensor.matmul(
                    out=bank[:, :],
                    lhsT=px_t[part0 : part0 + 48, 128 * p : 128 * (p + 1)],
                    rhs=x_t[part0 : part0 + 48, h0 : h0 + chunk],
                    start=True,
                    stop=False,
                    skip_group_check=True,
                )

            def emit_relu(c, i):
                """Relu for pipeline step i (i in 0..13); steps 12/13 are the
                fused [128,512] relus of banks (12,14) and (13,15)."""
                st = chunks[c]
                if i == 12:  # fused bank (12,14)
                    p = PAIR_OF[12][0]
                    h = hpool.tile([128, chunk], BF16, tag="h", name=f"hA_{c}")
                    nc.scalar.activation(h[:, :], st.banks[p][:, :], relu_f)
                    st.h[12] = h
                    st.h[14] = h
                    return
                if i == 13:  # fused bank (13,15)
                    p = PAIR_OF[13][0]
                    h = hpool.tile([128, chunk], BF16, tag="h", name=f"hB_{c}")
                    nc.vector.tensor_scalar_max(
                        out=h[:, :], in0=st.banks[p][:, :], scalar1=0.0
                    )
                    st.h[13] = h
                    st.h[15] = h
                    return
                p, r = PAIR_OF[i]
                part0 = _node_part0(i)
                h = hpool.tile([128, chunk], BF16, tag="h", name=f"h_{c}_{i}")
                st.h[i] = h
                src = st.banks[p][64 * r : 64 * (r + 1), :]
                dst = h[part0 : part0 + 64, :]
                if i % 2 == 0:
                    nc.scalar.activation(dst, src, relu_f)
                else:
                    nc.vector.tensor_scalar_max(out=dst, in0=src, scalar1=0.0)

            def emit_chain(c, i):
                if i > 12:
                    return
                st = chunks[c]
                part0 = _h_part0(i)
                cp, cr = PAIR_OF[i + 1]
                ca, cb = PAIRS[cp]
                closes = (i + 1 == cb) or (i + 1 == ca and not (1 <= cb <= 13))
                nc.tensor.matmul(
                    out=st.banks[cp][:, :],
                    lhsT=mc_t[part0 : part0 + 64, 128 * i : 128 * (i + 1)],
                    rhs=st.h[i][part0 : part0 + 64, :],
                    start=False,
                    stop=closes,
                    skip_group_check=True,
                )

            A_NODES = {0, 2, 4, 6, 8, 10, 12, 13}

            def make_collect_ops(c):
                st = chunks[c]
                bank_a = qpool_t0.tile([128, chunk], F32, tag="bA", name=f"bA_{c}")
                bank_b = qpool_t8.tile([128, chunk], F32, tag="bB", name=f"bB_{c}")

                def collect(i):
                    part0 = _h_part0(i)
                    in_a = i in A_NODES
                    ob = bank_a if in_a else bank_b
                    nc.tensor.matmul(
                        out=ob[:, :],
                        lhsT=cl_t[part0 : part0 + 64, 128 * i : 128 * (i + 1)],
                        rhs=st.h[i][part0 : part0 + 64, :],
                        start=(i <= 1),
                        stop=(i == 13) if in_a else (i == 15),
                        skip_group_check=True,
                    )
                    st.h[i] = None

                ops = [lambda i=i: collect(i) for i in range(I_DIM)]

                def finals():
                    o_a = opool.tile([16, chunk], F32, tag="oa", name=f"oa_{c}")
                    nc.scalar.activation(o_a[:, :], bank_a[0:16, :], copy_f)
                    o_t = opool.tile([16, chunk], F32, tag="o", name=f"o_{c}")
                    nc.vector.scalar_tensor_tensor(
                        out=o_t[:, :],
                        in0=bank_b[0:16, :],
                        scalar=b2_t[:, 0:1],
                        in1=o_a[:, :],
                        op0=add_op,
                        op1=add_op,
                    )
                    c0 = c * chunk
                    nc.sync.dma_start(out=out_d[:, c0 : c0 + chunk], in_=o_t[:, :])

                ops.append(finals)
                return ops

            # ---------- pipeline ----------
            # base for pair containing lower node a lands at step a-1
            base_step = {p: a - 1 for p, (a, b) in enumerate(PAIRS)}
            pending = []

            def slot_pos(k, T):
                q = T - STAG * k
                if q < 0:
                    return None, None
                j, i = divmod(q, I_STEPS)
                c = NSLOT * j + k
                return (c, i) if c < n_chunks else (None, None)

            # prologue: group-0 x DMAs in first-use order, while ~24 scratch
            # matmuls (garbage data, never read) warm the PE's HAM clock gate
            for p in (4, 0, 5, 1, 6, 2, 7, 3):
                emit_xdma_group(0, p)
            ws0 = pool_t0.tile([128, chunk], F32, tag="bank", name="warm0")
            ws8 = pool_t8.tile([128, chunk], F32, tag="bank", name="warm8")
            for w in range(24):
                nc.tensor.matmul(
                    out=(ws0 if w % 2 == 0 else ws8)[:, :],
                    lhsT=px_t[(w % 2) * 64 : (w % 2) * 64 + 48, 0:128],
                    rhs=px_t[(w % 2) * 64 : (w % 2) * 64 + 48, 0:chunk],
                    start=True, stop=True, skip_group_check=True,
                )

            max_T = I_STEPS * ((n_chunks + NSLOT - 1) // NSLOT) + STAG * NSLOT + 8
            for T in range(max_T):
                for k in range(NSLOT):
                    c, i = slot_pos(k, T)
                    if c is None:
                        continue
                    # x DMA prefetch: during even chunk c, step 6..13 emit
                    # group c//2 + 2's DMAs (two pairs per step); group 1
                    # lands during chunk 0 steps 2..5 (deferred from prologue)
                    if c == 0 and 2 <= i < 6 and n_chunks > 2:
                        pa, pb = ((4, 0), (5, 1), (6, 2), (7, 3))[i - 2]
                        emit_xdma_group(1, pa)
                        emit_xdma_group(1, pb)
                    if c % 2 == 0 and 6 <= i < 10:
                        g = c // 2 + 2
                        if 2 * g < n_chunks:
                            for p in (2 * (i - 6), 2 * (i - 6) + 1):
                                emit_xdma_group(g, p)
                    # JIT bases for this chunk / the slot's next chunk
                    if i == 0 and c < NSLOT:
                        for p in range(8):
                            if base_step[p] < 0:
                                emit_base(c, p)
                    for p in range(8):
                        if base_step[p] == i:
                            emit_base(c, p)
                        nxt = c + NSLOT
                        if nxt < n_chunks and base_step[p] < 0 \
                                and i == base_step[p] + I_STEPS:
                            emit_base(nxt, p)
                    emit_relu(c, i)
                    # paced fill: ~1 ready collect mm per slot turn keeps the
                    # PE busy through the relu->chain wait without draining
                    # the backlog dry (supply is ~3.6 ops/global step)
                    npop = 3 if len(pending) > 26 else (
                        2 if len(pending) > 12 else (1 if pending else 0))
                    for _ in range(npop):
                        if pending:
                            pending.pop(0)()
                    emit_chain(c, i)
                    if i == I_STEPS - 1:
                        pending.extend(make_collect_ops(c))
            while pending:
                pending.pop(0)()

    nc.compile()
    return nc


def prep_weights(noise_d, mu, sigma, Wc, W1, b1, W2, b2):
    theta = mu + np.log1p(np.exp(sigma)) * noise_d  # [4, 256]
    w_p = W1[:, 48, :]  # [16, 64]
    b1e = b1.copy()
    for i in range(1, 14):
        b1e[i] = b1[i] + w_p[i] * b2[i - 1]

    px = np.zeros((128, 128 * 8), np.float32)
    for p, (a, b) in enumerate(PAIRS):
        part0 = 0 if p < 4 else 64
        for r, node in enumerate((a, b)):
            cols = slice(128 * p + 64 * r, 128 * p + 64 * (r + 1))
            px[part0 + 0 : part0 + 10, cols] = (
                Wc[:, 16 * node : 16 * (node + 1)] @ W1[node, 0:16, :]
            )
            px[part0 + 10 : part0 + 14, cols] = (
                theta[:, 16 * node : 16 * (node + 1)] @ W1[node, 16:32, :]
            )
            px[part0 + 14, cols] = b1e[node]
            px[part0 + 16 + 16 * r : part0 + 32 + 16 * r, cols] = W1[node, 32:48, :]

    mc = np.zeros((128, 128 * 13), np.float32)
    for i in range(13):
        part0 = 0 if i % 2 == 0 else 64  # h position of parent i (i <= 12)
        cp, cr = PAIR_OF[i + 1]
        c0 = 128 * i + 64 * cr
        mc[part0 : part0 + 64, c0 : c0 + 64] = np.outer(W2[i], w_p[i + 1])

    cl = np.zeros((128, 128 * 16), np.float32)
    for i in range(16):
        part0 = 0 if i in (13,) or (i % 2 == 0 and i != 14) else 64
        cl[part0 : part0 + 64, 128 * i + i] = W2[i]

    return {
        "PX": px.astype(BF16_NP),
        "MC": mc.astype(BF16_NP),
        "CLW": cl.astype(BF16_NP),
        "B2": b2.reshape(16, 1).astype(np.float32),
    }


def prep_core_inputs(noise, input_c, input_d, c, b_s: int = B_S):
    b0, b1_ = c * b_s, (c + 1) * b_s
    s = np.zeros((16, b_s), np.float32)
    s[0:10] = input_c[b0:b1_].T
    s[10:14] = input_d[b0:b1_].T
    s[14] = 1.0
    nT = noise[b0:b1_].T
    xt = np.empty((48, 8 * b_s), np.float32)
    for p, (a, b) in enumerate(PAIRS):
        cols = slice(p * b_s, (p + 1) * b_s)
        xt[0:16, cols] = s
        xt[16:32, cols] = nT[16 * a : 16 * (a + 1)]
        xt[32:48, cols] = nT[16 * b : 16 * (b + 1)]
    return {"XT": xt.astype(BF16_NP)}


_NC_LOCK = threading.Lock()
_NC_CACHE = {}


def _get_nc():
    with _NC_LOCK:
        if "nc" not in _NC_CACHE:
            _NC_CACHE["nc"] = build_nc()
        return _NC_CACHE["nc"]


def kernel(noise, input_c, input_d, noise_d, mu, sigma, Wc, W1, b1, W2, b2):
    noise = np.asarray(noise, np.float32)
    input_c = np.asarray(input_c, np.float32)
    input_d = np.asarray(input_d, np.float32)
    w = prep_weights(
        np.asarray(noise_d, np.float32),
        np.asarray(mu, np.float32),
        np.asarray(sigma, np.float32),
        np.asarray(Wc, np.float32),
        np.asarray(W1, np.float32),
        np.asarray(b1, np.float32),
        np.asarray(W2, np.float32),
        np.asarray(b2, np.float32),
    )
    in_maps = []
    for c in range(N_CORES):
        m = prep_core_inputs(noise, input_c, input_d, c)
        m.update(w)
        in_maps.append(m)

    nc = _get_nc()
    res = run_bass_kernel_spmd(nc, in_maps, list(range(N_CORES)))
    out = np.concatenate(
        [res.results[c]["OUT"].T for c in range(N_CORES)], axis=0
    )
    return np.ascontiguousarray(out, np.float32)

